# revision 1
# baseline (speedup 1.0000x reference)
"""Trainium2 Bass kernel for: conv2d(16->64, 3x3, VALID) + bias -> min over
channels -> tanh(tanh()).  Input x [64,16,256,256] f32, output [64,1,254,254].

Strategy (per core, data-parallel over batch: 8 images/core):
  - Conv as matmuls with the x-patch (bf16) as the stationary operand and a
    block-Toeplitz weight matrix (bf16) moving, so conv output lands as
    [width-positions (partitions), rows*couts (free)] in f32 PSUM and the
    channel-min is a free-dim reduce.  Contraction K = 17 channels x 7 rows
    = 119 (channel 16 is a host-added ones-channel carrying the bias); a
    7-row window yields R=5 output rows per 3-matmul (dx) PSUM trio.
    bf16 + 128-wide stationary enables Fast Weight Load and halves the
    input DMA (x error ~2^-9 << the 2e-2 gate).
  - The DVE tensor_reduce(min) is capped at 1 elem/cycle (no 2x/4x uop) and
    alone cannot keep up with the PE (458ns/tile vs ~400ns production), so
    ~40% of the (w, jb) tiles are consumed by an offload chain instead:
    ScalarE copies PSUM->SBUF as bf16 (fold-friendly layout), GPSIMD does
    two tensor-tensor min folds (64->32->16 couts), DVE finishes with a
    cheap [*,5,16] reduce.  All engines stay under ~80% so the PE never
    stalls.
  - Min results land in bf16 staging [128 j, 256 rows]; epilogue: PE
    transpose (identity matmul), double-tanh on ScalarE from PSUM,
    contiguous row stores on the ACT HWDGE ring.
"""

import sys

for _p in ("/opt/trn_rl_repo", "/root/.axon_site/_ro/trn_rl_repo"):
    if _p not in sys.path:
        sys.path.insert(0, _p)

import numpy as np

B, CIN, H, W = 64, 16, 256, 256
COUT, KK = 64, 3
HO, WO = H - 2, W - 2  # 254
N_CORES = 8
B_LOC = B // N_CORES  # 8 images per core

# geometry
WIN_ROWS = 7          # input rows per window
R = WIN_ROWS - KK + 1  # 5 output rows per window
KDIM = (CIN + 1) * WIN_ROWS  # 119 contraction rows (incl. ones channel)
NDIM = R * COUT       # 320 moving free size
MJ = 128              # output width positions per j-block
J0S = (0, WO - MJ)    # j origin per block; cols 126/127 overlap benignly
N_JB = 2
N_WIN = 51            # windows: row0 = 5w for w<50, 249 for w=50
_cache = {}


def _is_b(w, jb):
    """Offload-chain tiles (ScalarE copy + DVE fold).  Measured as a net
    loss on hw (ACT copies run ~2x their modeled cost, 433us vs 405us), so
    disabled; kept behind the 'off' ablate flag for re-testing."""
    return False


def _build_wblocks(conv_weight, conv_bias):
    """wblk[dx][rho*17+ci, r*64+co] = W[co,ci,rho-r,dx]; bias on the ones-
    channel row (rho=0, ci=CIN) of dx=0.  Partition order matches the
    [B, H, C, W] host layout of x so the window DMA merges (row, chan)."""
    wblk = np.zeros((KK, KDIM, NDIM), dtype=np.float32)
    for dx in range(KK):
        for ci in range(CIN):
            for rho in range(WIN_ROWS):
                k = rho * (CIN + 1) + ci
                for r in range(R):
                    dy = rho - r
                    if 0 <= dy < KK:
                        wblk[dx, k, r * COUT:(r + 1) * COUT] = conv_weight[:, ci, dy, dx]
    k_bias = CIN  # (rho=0, ci=16)
    for r in range(R):
        wblk[0, k_bias, r * COUT:(r + 1) * COUT] = conv_bias
    return wblk


def _build_nc(reps=1, ablate=()):
    import concourse.bass as bass
    import concourse.bacc as bacc
    import concourse.tile as tile
    from concourse import mybir

    f32 = mybir.dt.float32
    bf16 = mybir.dt.bfloat16

    nc = bacc.Bacc(None)
    # x_aug host layout is [B, H, C, W]: window partitions are (row, chan)
    x_aug = nc.dram_tensor("x_aug", [B_LOC, H, CIN + 1, W], bf16, kind="ExternalInput")
    wblk_d = nc.dram_tensor("wblk", [KK, KDIM, NDIM], bf16, kind="ExternalInput")
    ident_d = nc.dram_tensor("ident", [MJ, MJ], bf16, kind="ExternalInput")
    y = nc.dram_tensor("y", [B_LOC, HO, WO], f32, kind="ExternalOutput")

    with tile.TileContext(nc) as tc:
        with (
            tc.tile_pool(name="consts", bufs=1) as consts,
            tc.tile_pool(name="wins", bufs=3) as wins,
            tc.tile_pool(name="stage", bufs=4) as stage,
            tc.tile_pool(name="fold", bufs=4) as fold,
            tc.tile_pool(name="outs", bufs=4) as outs,
            tc.tile_pool(name="cpsum", bufs=6, space="PSUM") as cpsum,
            tc.tile_pool(name="tpsum", bufs=2, space="PSUM") as tpsum,
        ):
            wblk_s = consts.tile([KDIM, KK, NDIM], bf16)
            nc.sync.dma_start(out=wblk_s[:], in_=wblk_d.rearrange("k d n -> d k n"))
            ident_s = consts.tile([MJ, MJ], bf16)
            nc.sync.dma_start(out=ident_s[:], in_=ident_d[:])

            import contextlib
            loop_ctx = tc.For_i(0, reps, 1) if reps > 1 else contextlib.nullcontext()
            with loop_ctx:
                _emit_body(nc, tc, bass, mybir, ablate, locals())
    nc.finalize()
    return nc


def _emit_body(nc, tc, bass, mybir, ablate, env):
    f32 = env["f32"]
    bf16 = env["bf16"]
    x_aug, y = env["x_aug"], env["y"]
    wblk_s, ident_s = env["wblk_s"], env["ident_s"]
    wins, stage, fold, outs = env["wins"], env["stage"], env["fold"], env["outs"]
    cpsum, tpsum = env["cpsum"], env["tpsum"]
    CW = (CIN + 1) * W  # elements per image row (all channels)
    MIN = mybir.AluOpType.min

    def _epilogue(b, stagings):
        for jb in range(N_JB):
            j0 = J0S[jb]
            for rb in range(2):
                ps_t = tpsum.tile([MJ, MJ], bf16, name="ps_t")
                nc.tensor.transpose(
                    out=ps_t[:], in_=stagings[jb][:, 128 * rb:128 * rb + MJ],
                    identity=ident_s[:],
                )
                t1 = outs.tile([MJ, MJ], f32, name="t1")
                nc.scalar.activation(
                    out=t1[:], in_=ps_t[:],
                    func=mybir.ActivationFunctionType.Tanh,
                )
                t2 = outs.tile([MJ, MJ], f32, name="t2")
                nc.scalar.activation(
                    out=t2[:], in_=t1[:],
                    func=mybir.ActivationFunctionType.Tanh,
                )
                nrows = 128 if rb == 0 else HO - 128  # 126 valid rows in rb=1
                nc.scalar.dma_start(
                    out=y[b, 128 * rb:128 * rb + nrows, j0:j0 + MJ],
                    in_=t2[0:nrows, 0:MJ],
                )

    pending = None  # defer each image's epilogue into the next image's
    for b in range(B_LOC):  # matmul stream so the in-order PE never waits
        bigx = wins.tile([KDIM, N_WIN, W], bf16, name="bigx")
        if "nodma" in ablate:
            nc.sync.dma_start(
                out=bigx[:, 0, :],
                in_=x_aug[b, 0:WIN_ROWS, :, :].rearrange("r c w -> (r c) w"),
            )
        else:
            # windows 0..49 (uniform row0 = 5w) in chunked DMAs; w=50 alone
            x_b = x_aug[b]
            step = 13 if "chunk13" in ablate else (50 if "chunk50" in ablate else 25)
            eng = nc.scalar if "actin" in ablate else nc.sync
            for w_lo in range(0, 50, step):
                nw = min(step, 50 - w_lo)
                src = bass.AP(
                    tensor=x_b.tensor,
                    offset=x_b.offset + 5 * w_lo * CW,
                    ap=[[CW, WIN_ROWS], [W, CIN + 1], [5 * CW, nw], [1, W]],
                )
                eng.dma_start(out=bigx[:, w_lo:w_lo + nw, :], in_=src)
            eng.dma_start(
                out=bigx[:, N_WIN - 1, :],
                in_=x_aug[b, HO - R:H, :, :].rearrange("r c w -> (r c) w"),
            )
        stagings = []
        for jb in range(N_JB):
            staging = stage.tile([MJ, 256], bf16, name=f"staging{jb}", tag=f"st{jb}")
            stagings.append(staging)
        for w in range(N_WIN):
            row0 = 5 * w if w < N_WIN - 1 else HO - R
            for jb in range(N_JB):
                j0 = J0S[jb]
                psum = cpsum.tile([MJ, NDIM], f32, name="psum")
                if "nomm" not in ablate:
                    wi = 0 if "nodma" in ablate else w
                    for dx in range(KK):
                        nc.tensor.matmul(
                            out=psum[:],
                            lhsT=bigx[:, wi, j0 + dx:j0 + dx + MJ],
                            rhs=wblk_s[:, dx, :],
                            start=(dx == 0),
                            stop=(dx == KK - 1),
                        )
                if "nodve" in ablate:
                    continue
                if ("off" in ablate or _is_b(w, jb)) and "nooff" not in ablate:
                    # offload chain: ACT cast-copy to bf16, then a packed
                    # 2x-mode DVE fold (tensor_tensor min has a 2x_1p uop;
                    # tensor_reduce is 1x-only) halves the reduce input
                    sb = fold.tile([MJ, 2, R, 32], bf16, name="sb")
                    nc.scalar.activation(
                        out=sb.rearrange("p c2 r c -> p r c2 c"),
                        in_=psum.rearrange("p (r c2 c) -> p r c2 c", c2=2, c=32),
                        func=mybir.ActivationFunctionType.Copy,
                    )
                    f1 = fold.tile([MJ, R, 32], bf16, name="f1")
                    nc.vector.tensor_tensor(
                        out=f1[:], in0=sb[:, 0], in1=sb[:, 1], op=MIN,
                    )
                    nc.vector.tensor_reduce(
                        out=stagings[jb][:, row0:row0 + R],
                        in_=f1[:],
                        axis=mybir.AxisListType.X,
                        op=MIN,
                    )
                else:
                    nc.vector.tensor_reduce(
                        out=stagings[jb][:, row0:row0 + R],
                        in_=psum.rearrange("p (r c) -> p r c", c=COUT),
                        axis=mybir.AxisListType.X,
                        op=MIN,
                    )
            w_defer = 2 if "defer2" in ablate else (12 if "defer12" in ablate else 6)
            if w == w_defer and pending is not None and "noepi" not in ablate:
                _epilogue(*pending)
                pending = None
        if "nodefer" in ablate and "noepi" not in ablate:
            _epilogue(b, stagings)
        else:
            pending = (b, stagings)
    if pending is not None and "noepi" not in ablate:
        _epilogue(*pending)


def _get_compiled(reps=1, ablate=()):
    key = ("nc", reps, tuple(ablate))
    if key not in _cache:
        _cache[key] = _build_nc(reps, ablate)
    return _cache[key]


def _to_bf16(a):
    import ml_dtypes
    return np.asarray(a, dtype=np.float32).astype(ml_dtypes.bfloat16)


def make_in_maps(x, conv_weight, conv_bias):
    x = np.asarray(x, dtype=np.float32)
    x_aug = np.empty((B, H, CIN + 1, W), dtype=np.float32)
    x_aug[:, :, :CIN] = x.transpose(0, 2, 1, 3)
    x_aug[:, :, CIN] = 1.0
    x_aug = _to_bf16(x_aug)
    wblk = _to_bf16(_build_wblocks(
        np.asarray(conv_weight, dtype=np.float32),
        np.asarray(conv_bias, dtype=np.float32)))
    ident = _to_bf16(np.eye(MJ, dtype=np.float32))
    return [
        {
            "x_aug": np.ascontiguousarray(x_aug[c * B_LOC:(c + 1) * B_LOC]),
            "wblk": wblk,
            "ident": ident,
        }
        for c in range(N_CORES)
    ]


def kernel(x, conv_weight, conv_bias):
    from concourse.bass_utils import run_bass_kernel_spmd

    nc = _get_compiled()
    in_maps = make_in_maps(x, conv_weight, conv_bias)
    res = run_bass_kernel_spmd(nc, in_maps, core_ids=list(range(N_CORES)))
    out = np.concatenate([res.results[c]["y"] for c in range(N_CORES)], axis=0)
    return out.reshape(B, 1, HO, WO)



# revision 6
# speedup vs baseline: 4.3626x; 4.3626x over previous
"""Trainium2 Bass kernel for: conv2d(16->64, 3x3, VALID) + bias -> min over
channels -> tanh(tanh()).  Input x [64,16,256,256] f32, output [64,1,254,254].

Strategy (per core, data-parallel over batch: 8 images/core):
  - Conv as matmuls with the x-patch (bf16) as the stationary operand and a
    block-Toeplitz weight matrix (bf16) moving: conv output lands as
    [width-positions (partitions), rows*couts (free)] in f32 PSUM and the
    channel-min is a free-dim reduce.  6-row windows give R=4 output rows
    per 3-matmul (dx) group; TWO consecutive windows pack into ONE 2KB PSUM
    bank ([128, 2, 4, 64] = 512 f32), so every drain op sees a full bank
    and the per-op fixed costs (120 cyc PSUM init on DVE, 172 on ACT)
    amortize over 8 output rows instead of 5.
  - The channel-min drains 33M f32 PSUM elements/core -- more than either
    DVE or ACT alone can move at the PE's pace -- so bank-tiles alternate
    between two pipelines: path A = direct DVE tensor_reduce; path B = ACT
    copy PSUM->SBUF bf16 in a half-split layout, one flat DVE
    tensor_tensor min fold (2x-mode eligible) and a half-size DVE reduce.
    OFF_PAT tunes the fraction so DVE and ACT finish together.
  - Min results land in bf16 staging [128 j, 256 rows]; the epilogue is
    PE-free: ACT double-tanh straight off staging, DMA out in transposed
    [jb, j, row] layout, and the host reassembles with cheap numpy
    transposes -- saving the PE transposes and freeing 2 PSUM banks.
"""

import sys

for _p in ("/opt/trn_rl_repo", "/root/.axon_site/_ro/trn_rl_repo"):
    if _p not in sys.path:
        sys.path.insert(0, _p)

import numpy as np

B, CIN, H, W = 64, 16, 256, 256
COUT, KK = 64, 3
HO, WO = H - 2, W - 2  # 254
N_CORES = 8
B_LOC = B // N_CORES  # 8 images per core

# geometry
WIN_ROWS = 6          # input rows per window
R = WIN_ROWS - KK + 1  # 4 output rows per window
KDIM = (CIN + 1) * WIN_ROWS  # 102 contraction rows (incl. ones channel)
NDIM = R * COUT       # 256 moving free size
MJ = 128              # output width positions per j-block
J0S = (0, WO - MJ)    # j origin per block; cols 126/127 overlap benignly
N_JB = 2
N_WIN = 64            # windows: row0 = 4w for w<63, 250 for w=63
N_PAIR = N_WIN // 2   # two windows share one PSUM bank
_cache = {}


def _row0(w):
    return 4 * w if w < N_WIN - 1 else HO - R  # 250


# Bank-tile t uses path B (ACT copy + flat DVE fold) when OFF_PAT[t % len]
# else path A (direct DVE reduce).  Measured per-bank costs: A = DVE 791ns;
# B = ACT 791ns + DVE 644ns (fold 254 + reduce 390).  Balance at ~13/16 B.
OFF_PAT = (False, True, True, True, True, True, False, True, True, True,
           True, False, True, True, True, True)


def _build_wblocks(conv_weight, conv_bias):
    """wblk[dx][rho*17+ci, r*64+co] = W[co,ci,rho-r,dx]; bias on the ones-
    channel row (rho=0, ci=CIN) of dx=0.  Partition order matches the
    [B, H, C, W] host layout of x so the window DMA merges (row, chan)."""
    wblk = np.zeros((KK, KDIM, NDIM), dtype=np.float32)
    for dx in range(KK):
        for ci in range(CIN):
            for rho in range(WIN_ROWS):
                k = rho * (CIN + 1) + ci
                for r in range(R):
                    dy = rho - r
                    if 0 <= dy < KK:
                        wblk[dx, k, r * COUT:(r + 1) * COUT] = conv_weight[:, ci, dy, dx]
    k_bias = CIN  # (rho=0, ci=16)
    for r in range(R):
        wblk[0, k_bias, r * COUT:(r + 1) * COUT] = conv_bias
    return wblk


def _build_nc(reps=1, ablate=()):
    import concourse.bass as bass
    import concourse.bacc as bacc
    import concourse.tile as tile
    from concourse import mybir

    f32 = mybir.dt.float32
    bf16 = mybir.dt.bfloat16

    nc = bacc.Bacc(None)
    # x_aug host layout is [B, H, C, W]: window partitions are (row, chan)
    x_aug = nc.dram_tensor("x_aug", [B_LOC, H, CIN + 1, W], bf16, kind="ExternalInput")
    wblk_d = nc.dram_tensor("wblk", [KK, KDIM, NDIM], bf16, kind="ExternalInput")
    # output in transposed layout [img, jb, j, row]; host reassembles
    y = nc.dram_tensor("y", [B_LOC, N_JB, MJ, HO], f32, kind="ExternalOutput")

    with tile.TileContext(nc) as tc:
        with (
            tc.tile_pool(name="consts", bufs=1) as consts,
            tc.tile_pool(name="wins", bufs=3) as wins,
            tc.tile_pool(name="stage", bufs=4) as stage,
            tc.tile_pool(name="fold", bufs=4) as fold,
            tc.tile_pool(name="outs", bufs=4) as outs,
            tc.tile_pool(name="cpsum", bufs=8, space="PSUM") as cpsum,
        ):
            wblk_s = consts.tile([KDIM, KK, NDIM], bf16)
            nc.sync.dma_start(out=wblk_s[:], in_=wblk_d.rearrange("k d n -> d k n"))

            import contextlib
            loop_ctx = tc.For_i(0, reps, 1) if reps > 1 else contextlib.nullcontext()
            with loop_ctx:
                _emit_body(nc, tc, bass, mybir, ablate, locals())
    nc.finalize()
    return nc


def _emit_body(nc, tc, bass, mybir, ablate, env):
    f32 = env["f32"]
    bf16 = env["bf16"]
    x_aug, y = env["x_aug"], env["y"]
    wblk_s = env["wblk_s"]
    wins, stage, fold, outs = env["wins"], env["stage"], env["fold"], env["outs"]
    cpsum = env["cpsum"]
    CW = (CIN + 1) * W  # elements per image row (all channels)
    MIN = mybir.AluOpType.min

    def _stg_out(stg, p):
        """Staging view [2, 4] for pair p's 8 output rows.  Pairs 0..30 are
        contiguous (rows 8p..8p+7); the last pair overlaps benignly (rows
        248..251 and 250..253 -- row 250/251 written twice, same value)."""
        r0a, r0b = _row0(2 * p), _row0(2 * p + 1)
        return bass.AP(
            tensor=stg.tensor,
            offset=stg.offset + r0a,
            ap=[list(stg.ap[0]), [r0b - r0a, 2], [1, R]],
        )

    def _epilogue(b, stagings):
        for jb in range(N_JB):
            t1 = outs.tile([MJ, HO], f32, name="t1")
            nc.scalar.activation(
                out=t1[:], in_=stagings[jb][:, 0:HO],
                func=mybir.ActivationFunctionType.Tanh,
            )
            t2 = outs.tile([MJ, HO], f32, name="t2")
            nc.scalar.activation(
                out=t2[:], in_=t1[:],
                func=mybir.ActivationFunctionType.Tanh,
            )
            nc.scalar.dma_start(out=y[b, jb], in_=t2[:])

    for b in range(B_LOC):
        bigx = wins.tile([KDIM, N_WIN, W], bf16, name="bigx")
        # windows 0..62 (uniform row0 = 4w) in chunked DMAs; w=63 alone
        x_b = x_aug[b]
        step = 21
        for w_lo in range(0, N_WIN - 1, step):
            nw = min(step, N_WIN - 1 - w_lo)
            src = bass.AP(
                tensor=x_b.tensor,
                offset=x_b.offset + 4 * w_lo * CW,
                ap=[[CW, WIN_ROWS], [W, CIN + 1], [4 * CW, nw], [1, W]],
            )
            nc.sync.dma_start(out=bigx[:, w_lo:w_lo + nw, :], in_=src)
        nc.sync.dma_start(
            out=bigx[:, N_WIN - 1, :],
            in_=x_aug[b, HO - R:H, :, :].rearrange("r c w -> (r c) w"),
        )

        stagings = []
        for jb in range(N_JB):
            staging = stage.tile([MJ, 256], bf16, name=f"staging{jb}", tag=f"st{jb}")
            stagings.append(staging)
        for p in range(N_PAIR):
            for jb in range(N_JB):
                j0 = J0S[jb]
                t = p * N_JB + jb
                psum = cpsum.tile([MJ, 2, NDIM], f32, name="psum")
                for u in range(2):
                    w = 2 * p + u
                    for dx in range(KK):
                        nc.tensor.matmul(
                            out=psum[:, u],
                            lhsT=bigx[:, w, j0 + dx:j0 + dx + MJ],
                            rhs=wblk_s[:, dx, :],
                            start=(dx == 0),
                            stop=(dx == KK - 1),
                        )
                offload = OFF_PAT[t % len(OFF_PAT)] and "nooff" not in ablate
                stg_view = _stg_out(stagings[jb], p)
                if offload:
                    # path B: ACT drains the bank as bf16 with cout-halves
                    # split to the outer axis; one flat 2x DVE fold then a
                    # half-size reduce
                    lb = fold.tile([MJ, 2, 2, R, 32], bf16, name="lb")
                    nc.scalar.activation(
                        out=lb.rearrange("p c2 u r c -> p u r c2 c"),
                        in_=psum.rearrange("p u (r c2 c) -> p u r c2 c",
                                           c2=2, c=32),
                        func=mybir.ActivationFunctionType.Copy,
                    )
                    lflat = lb.rearrange("p c2 u r c -> p (c2 u r c)")
                    g = fold.tile([MJ, 2, R, 32], bf16, name="g")
                    nc.vector.tensor_tensor(
                        out=g.rearrange("p u r c -> p (u r c)"),
                        in0=lflat[:, 0:2 * R * 32],
                        in1=lflat[:, 2 * R * 32:4 * R * 32],
                        op=MIN,
                    )
                    nc.vector.tensor_reduce(
                        out=stg_view,
                        in_=g[:],
                        axis=mybir.AxisListType.X,
                        op=MIN,
                    )
                else:
                    # path A: direct DVE reduce from the full PSUM bank
                    nc.vector.tensor_reduce(
                        out=stg_view,
                        in_=psum.rearrange("p u (r c) -> p u r c", c=COUT),
                        axis=mybir.AxisListType.X,
                        op=MIN,
                    )
        _epilogue(b, stagings)


def _get_compiled(reps=1, ablate=()):
    key = ("nc", reps, tuple(ablate))
    if key not in _cache:
        _cache[key] = _build_nc(reps, ablate)
    return _cache[key]


def _to_bf16(a):
    import ml_dtypes
    return np.asarray(a, dtype=np.float32).astype(ml_dtypes.bfloat16)


def make_in_maps(x, conv_weight, conv_bias):
    x = np.asarray(x, dtype=np.float32)
    x_aug = np.empty((B, H, CIN + 1, W), dtype=np.float32)
    x_aug[:, :, :CIN] = x.transpose(0, 2, 1, 3)
    x_aug[:, :, CIN] = 1.0
    x_aug = _to_bf16(x_aug)
    wblk = _to_bf16(_build_wblocks(
        np.asarray(conv_weight, dtype=np.float32),
        np.asarray(conv_bias, dtype=np.float32)))
    return [
        {
            "x_aug": np.ascontiguousarray(x_aug[c * B_LOC:(c + 1) * B_LOC]),
            "wblk": wblk,
        }
        for c in range(N_CORES)
    ]


def kernel(x, conv_weight, conv_bias):
    from concourse.bass_utils import run_bass_kernel_spmd

    nc = _get_compiled()
    in_maps = make_in_maps(x, conv_weight, conv_bias)
    res = run_bass_kernel_spmd(nc, in_maps, core_ids=list(range(N_CORES)))
    out = np.empty((B, 1, HO, WO), dtype=np.float32)
    for c in range(N_CORES):
        yc = res.results[c]["y"]  # [B_LOC, 2, MJ, HO]
        blk = out[c * B_LOC:(c + 1) * B_LOC, 0]
        blk[:, :, 0:MJ] = yc[:, 0].transpose(0, 2, 1)
        blk[:, :, J0S[1]:WO] = yc[:, 1].transpose(0, 2, 1)
    return out
